# revision 21
# baseline (speedup 1.0000x reference)
"""Capacity-based MoE router for Trainium2 (8 NeuronCores, data-parallel).

Device kernel (per core): logits_shard.T [8, 2048] = gate_w @ x_shard.T.
Streams the core's 32MB slice of x once (memory-roofline work), fp32 matmul
accumulated in PSUM.

Host: top-k / softmax / capacity ranking / aux losses on the tiny [16384, 8]
logits, mirroring the jax reference ops exactly in numpy fp32.
"""

import numpy as np
from contextlib import ExitStack

import concourse.bass as bass
import concourse.bacc as bacc
import concourse.mybir as mybir
from concourse.tile import TileContext
from concourse.bass_utils import run_bass_kernel_spmd

N_TOKENS = 16384
D_MODEL = 4096
NUM_EXPERTS = 8
TOP_K = 2
CAPACITY = N_TOKENS // NUM_EXPERTS  # 2048
N_CORES = 8
TPC = N_TOKENS // N_CORES  # tokens per core = 2048

F32 = mybir.dt.float32
KT = D_MODEL // 128  # 32 contraction tiles
NB = TPC // 512      # 4 moving-operand chunks (N<=512 fp32)

_CACHE = {}


def _build_bass(repeat=1):
    key = ("nc", repeat)
    if key in _CACHE:
        return _CACHE[key]
    # Bacc (not raw Bass): its finalize() runs generate_event_semaphores,
    # which splits multi-sem waits — TRN2 instructions fit only one.
    nc = bacc.Bacc()
    xt = nc.dram_tensor("xt", [D_MODEL, TPC], F32, kind="ExternalInput")
    wt = nc.dram_tensor("wt", [D_MODEL, NUM_EXPERTS], F32, kind="ExternalInput")
    out = nc.dram_tensor("logits_t", [NUM_EXPERTS, TPC], F32, kind="ExternalOutput")

    with ExitStack() as ctx:
        tc = ctx.enter_context(TileContext(nc))
        xpool = ctx.enter_context(tc.tile_pool(name="x", bufs=4))
        wpool = ctx.enter_context(tc.tile_pool(name="w", bufs=1))
        opool = ctx.enter_context(tc.tile_pool(name="o", bufs=2))
        ppool = ctx.enter_context(tc.tile_pool(name="p", bufs=2, space="PSUM"))

        # gate weights, all 32 k-chunks resident: [128, kt, 8]
        wtile = wpool.tile([128, KT, NUM_EXPERTS], F32)
        nc.gpsimd.dma_start(
            out=wtile[:], in_=wt.rearrange("(kt p) e -> p kt e", p=128)
        )

        # x loads grouped GROUP k-chunks per DMA: 4MB transfers.
        GROUP = 4
        NGRP = KT // GROUP
        xt_v = xt.rearrange("(g j p) n -> p g j n", j=GROUP, p=128)
        for _rep in range(repeat):
            psum = ppool.tile([NUM_EXPERTS, TPC], F32, tag="psum")
            for g in range(NGRP):
                xtile = xpool.tile([128, GROUP, TPC], F32, tag="x")
                nc.gpsimd.dma_start(out=xtile[:], in_=xt_v[:, g, :, :])
                for j in range(GROUP):
                    kt = g * GROUP + j
                    for nb in range(NB):
                        nc.tensor.matmul(
                            psum[:, nb * 512:(nb + 1) * 512],
                            lhsT=wtile[:, kt, :],
                            rhs=xtile[:, j, nb * 512:(nb + 1) * 512],
                            start=(kt == 0),
                            stop=(kt == KT - 1),
                        )

            otile = opool.tile([NUM_EXPERTS, 2, TPC // 2], F32, tag="o")
            nc.vector.tensor_copy(
                otile[:], psum[:].rearrange("e (a b) -> e a b", a=2)
            )
            nc.gpsimd.dma_start(
                out=out.rearrange("e (a b) -> e a b", a=2), in_=otile[:]
            )

    # run_bass_via_pjrt does not finalize; Bacc needs it (register alloc,
    # sync-wait splitting).
    nc.finalize()
    _CACHE[key] = nc
    return nc


def _device_logits(x, gate_w, trace=False):
    """Run the Bass kernel on 8 cores; return full [N_TOKENS, 8] f32 logits."""
    nc = _build_bass()
    wt = np.ascontiguousarray(gate_w.T).astype(np.float32, copy=False)
    in_maps = []
    for c in range(N_CORES):
        xs = x[c * TPC:(c + 1) * TPC, :]
        in_maps.append({"xt": np.ascontiguousarray(xs.T), "wt": wt})
    res = run_bass_kernel_spmd(nc, in_maps, list(range(N_CORES)), trace=trace)
    logits = np.empty((N_TOKENS, NUM_EXPERTS), np.float32)
    for c in range(N_CORES):
        logits[c * TPC:(c + 1) * TPC, :] = res.results[c]["logits_t"].T
    _CACHE["last_exec_time_ns"] = res.exec_time_ns
    _CACHE["last_results"] = res
    return logits


def _postprocess(logits):
    """Numpy fp32 mirror of the reference's post-matmul ops."""
    l = logits.astype(np.float32)
    order = np.argsort(-l, axis=-1, kind="stable")
    top_k_indices = order[:, :TOP_K].astype(np.int32)
    top_k_logits = np.take_along_axis(l, top_k_indices, axis=-1)
    m = top_k_logits.max(axis=-1, keepdims=True)
    e = np.exp(top_k_logits - m, dtype=np.float32)
    p = e / e.sum(axis=-1, keepdims=True, dtype=np.float32)
    p = p / p.sum(axis=-1, keepdims=True, dtype=np.float32)

    flat_idx = top_k_indices.reshape(-1)
    flat_prob = p.reshape(-1).astype(np.float32)
    keep = np.zeros(N_TOKENS * TOP_K, dtype=bool)
    for eid in range(NUM_EXPERTS):
        memb = flat_idx == eid
        scores = np.where(memb, flat_prob, -np.inf).astype(np.float32)
        o = np.argsort(-scores, kind="stable")
        ranks = np.empty_like(o)
        ranks[o] = np.arange(N_TOKENS * TOP_K)
        keep |= (ranks < CAPACITY) & memb
    keep = keep.reshape(N_TOKENS, TOP_K)

    tki = (top_k_indices * keep.astype(np.int32)).astype(np.int32)
    tkp = (p * keep.astype(np.float32)).astype(np.float32)

    usage = np.zeros(NUM_EXPERTS, dtype=np.float32)
    fi = tki.reshape(-1)
    for eid in range(NUM_EXPERTS):
        usage[eid] = np.float32((fi == eid).sum())
    ideal = N_TOKENS * TOP_K / NUM_EXPERTS
    lbl = np.float32(np.mean((usage - np.float32(ideal)) ** 2, dtype=np.float32))

    mm = l.max(axis=-1)
    lse = mm + np.log(
        np.sum(np.exp(l - mm[:, None], dtype=np.float32), axis=-1, dtype=np.float32),
        dtype=np.float32,
    )
    zl = np.float32(np.mean(lse.astype(np.float32) ** 2, dtype=np.float32))
    return tki, tkp, lbl, zl, usage


def kernel(x, gate_w, _trace=False):
    x = np.asarray(x, dtype=np.float32)
    gate_w = np.asarray(gate_w, dtype=np.float32)
    logits = _device_logits(x, gate_w, trace=_trace)
    return _postprocess(logits)


# ---------------- benchmarking (test.py only, not used for grading) --------

def _timed_exec(nc, xt_concat, wt_concat, iters, warmup=3):
    """Execute nc's program on 8 cores with device-resident inputs; return
    min wall seconds per dispatch."""
    import time
    import jax
    from concourse.bass2jax import _bass_exec_p, install_neuronx_cc_hook
    from jax.experimental.shard_map import shard_map
    from jax.sharding import Mesh, PartitionSpec, NamedSharding
    import concourse.mybir as _mb

    install_neuronx_cc_hook()
    in_names, out_names, out_avals = [], [], []
    for alloc in nc.m.functions[0].allocations:
        if not isinstance(alloc, _mb.MemoryLocationSet):
            continue
        name = alloc.memorylocations[0].name
        if alloc.kind == "ExternalInput":
            in_names.append(name)
        elif alloc.kind == "ExternalOutput":
            out_names.append(name)
            out_avals.append(
                jax.core.ShapedArray(tuple(alloc.tensor_shape), _mb.dt.np(alloc.dtype))
            )
    n_params = len(in_names)
    all_names = in_names + out_names

    def _body(*args):
        outs = _bass_exec_p.bind(
            *args,
            out_avals=tuple(out_avals),
            in_names=tuple(all_names),
            out_names=tuple(out_names),
            lowering_input_output_aliases=(),
            sim_require_finite=True,
            sim_require_nnan=True,
            nc=nc,
        )
        return tuple(outs)

    devices = jax.devices()[:N_CORES]
    mesh = Mesh(np.asarray(devices), ("core",))
    spec = PartitionSpec("core")
    n_outs = len(out_names)
    sharded = jax.jit(
        shard_map(
            _body, mesh=mesh,
            in_specs=(spec,) * (n_params + n_outs),
            out_specs=(spec,) * n_outs,
            check_rep=False,
        ),
        donate_argnums=tuple(range(n_params, n_params + n_outs)),
        keep_unused=True,
    )
    sh = NamedSharding(mesh, spec)
    per_name = {"xt": xt_concat, "wt": wt_concat}
    if nc.dbg_addr is not None:
        per_name[nc.dbg_addr.name] = np.zeros((N_CORES * 1, 2), np.uint32)
    if nc.partition_id_tensor is not None:
        pshape = tuple(nc.partition_id_tensor.shape)
        pid = np.concatenate(
            [np.full(pshape, c, dtype=np.uint32) for c in range(N_CORES)], axis=0
        )
        per_name[nc.partition_id_tensor.name] = pid
    ins = [jax.device_put(per_name[n], sh) for n in in_names]
    zeros = [
        np.zeros((N_CORES * a.shape[0], *a.shape[1:]), a.dtype) for a in out_avals
    ]
    times = []
    for i in range(warmup + iters):
        zs = [z.copy() for z in zeros]
        t0 = time.perf_counter()
        outs = sharded(*ins, *zs)
        jax.block_until_ready(outs)
        dt = time.perf_counter() - t0
        if i >= warmup:
            times.append(dt)
    return min(times)


def measure_hw_time_ns(x, gate_w, r_lo=1, r_hi=16, iters=60):
    """Estimate on-device kernel time by slope: repeat the whole pipeline R
    times inside one NEFF; dispatch overhead cancels in t(r_hi) - t(r_lo)."""
    x = np.asarray(x, dtype=np.float32)
    gate_w = np.asarray(gate_w, dtype=np.float32)
    wt = np.ascontiguousarray(gate_w.T)
    xt_concat = np.concatenate(
        [np.ascontiguousarray(x[c * TPC:(c + 1) * TPC].T) for c in range(N_CORES)],
        axis=0,
    )
    wt_concat = np.concatenate([wt] * N_CORES, axis=0)
    t = {}
    for r in (r_lo, r_hi):
        nc = _build_bass(repeat=r)
        t[r] = _timed_exec(nc, xt_concat, wt_concat, iters)
    ns = (t[r_hi] - t[r_lo]) / (r_hi - r_lo) * 1e9
    return ns, t
